# revision 26
# baseline (speedup 1.0000x reference)
"""Trainium2 Bass kernel for multi-head attention graph scatter.

Computes, for each of 8 heads h (one NeuronCore per head):
    q_h = query @ w_q[:, h*32:(h+1)*32]          # [3000, 32]
    k_h = key_emb @ w_k[:, h*32:(h+1)*32]        # [4096, 32]
    attn_h = softmax(q_h @ k_h.T / sqrt(32))     # [3000, 4096]
    graphs[h, qt, :] = attn_h                    # [4096, 4096], rest zeros

The rel-err budget (2e-2; this kernel lands ~1e-3) is spent on
bandwidth: inputs are uploaded fp16 and pre-transposed (host-side layout
marshaling so device loads are plain packed DMAs), the PE runs fp16
matmuls, and the [3000, 4096] attention block is stored fp16 — halving
the dominant HBM write traffic vs f32. The weights plus the first 512
columns of both transposed inputs ride in a single "boot" upload so the
first exp chunk only waits on one DMA + one projection chain.

The device emits the softmax in scaled form: unnormalized exp(s) tiles
(fp16) plus the per-row f32 sums it computed on the DVE. Each exp chunk
is DMA'd the moment the ACT engine produces it, so the store stream runs
at exp pace and nothing waits on the normalization. The host divides by
the sums during the fp16 -> f32 conversion + row scatter it performs
anyway.

Per-core engine budget (cost model, 106.4us total): ACT exp 93.2us
busy (the bottleneck), DMA 78.5us, DVE 58.6us (row-sum trees +
projection copies; GPSIMD cannot read PSUM so the copies live on DVE),
PE 50us, Pool 42us (alternate-tile tree first passes).
"""

import math
import sys

import numpy as np

if "/opt/trn_rl_repo" not in sys.path:
    sys.path.insert(0, "/opt/trn_rl_repo")

N_HEAD = 8
D_K = 32
CONCEPT_NUM = 4096
MASK_NUM = 3000
INPUT_DIM = 256

P = 128  # SBUF partitions
MPAD = 3008  # mask columns padded (transposed-query upload width)
BOOTW = 4 * D_K + 2 * 512  # boot tensor columns: w_q|w_k halves + query piece 0

_BUILD_CACHE = {}


def _build_module():
    """Build the per-core Bass module (identical on all 8 cores; inputs differ)."""
    import concourse.bacc as bacc
    import concourse.mybir as mybir
    import concourse.tile as tile

    f32 = mybir.dt.float32
    f16 = mybir.dt.float16
    SCALE = 1.0 / math.sqrt(D_K)
    ADD = mybir.AluOpType.add
    EXP = mybir.ActivationFunctionType.Exp

    nc = bacc.Bacc("TRN2", target_bir_lowering=False, debug=False, num_devices=N_HEAD)

    # inputs are uploaded transposed: queryT[c, i] = query[i, c] etc.
    # boot packs [wq_a0 | wq_a1 | wk_a0 | wk_a1 | qT_a0[:512] | qT_a1[:512]];
    # every other input piece rides one paired DMA per piece (both row-halves
    # via a (a p) c -> p a c rearrange), halving HWDGE stages and sems.
    boot_d = nc.dram_tensor("boot", [P, BOOTW], f16, kind="ExternalInput")
    queryT_d = nc.dram_tensor("queryT", [INPUT_DIM, MPAD], f16, kind="ExternalInput")
    keyT_d = nc.dram_tensor("keyT", [INPUT_DIM, CONCEPT_NUM], f16, kind="ExternalInput")
    attn = nc.dram_tensor("attn", [MASK_NUM, CONCEPT_NUM], f16, kind="ExternalOutput")
    sums_d = nc.dram_tensor("sums", [P, 24], f32, kind="ExternalOutput")

    m_tiles = [P] * (MASK_NUM // P) + ([MASK_NUM % P] if MASK_NUM % P else [])
    n_mt = len(m_tiles)  # 24 (23 full + 56-row tail)
    N1 = 4  # tiles whose A-half is emitted before the key tail arrives
    POOL_LAST = 19  # no pool tree-assists after this tile (drain latency)

    # load piece column ranges (start, width); query piece 0 is in boot.
    KP = [(0, 512), (512, 512), (1024, 1024), (2048, 2048)]
    QP = [(512, 512), (1024, 1024), (2048, 960)]

    with tile.TileContext(nc) as tc:
        with (
            tc.tile_pool(name="const", bufs=1) as const_pool,
            tc.tile_pool(name="trans", bufs=1) as trans_pool,
            tc.tile_pool(name="proj", bufs=1) as proj_pool,
            tc.tile_pool(name="expp", bufs=7) as expp,
            tc.tile_pool(name="scr", bufs=2) as scrp,
            tc.tile_pool(name="stats", bufs=3) as stats,
            tc.tile_pool(name="spsum", bufs=2, space="PSUM") as spsum,
        ):
            boot = const_pool.tile([P, BOOTW], f16, name="boot")
            nc.sync.dma_start(boot[:], boot_d.ap())
            wq = (boot[:, 0:D_K], boot[:, D_K : 2 * D_K])
            wk = (boot[:, 2 * D_K : 3 * D_K], boot[:, 3 * D_K : 4 * D_K])
            O = 4 * D_K
            bq = (boot[:, O : O + 512], boot[:, O + 512 : O + 1024])

            warm = const_pool.tile([D_K, 256], f16, name="warm")
            nc.vector.memset(warm[:], 0.0)
            sums_sb = const_pool.tile([P, 24], f32, name="sums_sb")

            def psum_tile(nm):
                return spsum.tile([P, 2048], f32, tag="sc", name=nm)

            # ramp the PE clock before the first real matmuls
            for r in range(12):
                wps = psum_tile(f"warm{r}")
                nc.tensor.matmul(wps[:, :256], warm[:, :P], warm[:], start=True, stop=True)

            # --- fp16 staging tiles: [p, a, c] pairs, one DMA per piece ---
            keyP = [
                trans_pool.tile([P, 2, w], f16, name=f"keyP_{pc}")
                for pc, (c0, w) in enumerate(KP)
            ]
            queryP = [
                trans_pool.tile([P, 2, w], f16, name=f"queryP_{pc}")
                for pc, (c0, w) in enumerate(QP)
            ]
            # projected tensors, grouped to match load pieces
            kTg = [
                proj_pool.tile([D_K, 512], f16, name="kT_0"),
                proj_pool.tile([D_K, 512], f16, name="kT_1"),
                proj_pool.tile([D_K, 1024], f16, name="kT_23"),
                proj_pool.tile([D_K, 2048], f16, name="kT_47"),
            ]
            qTg = [
                proj_pool.tile([D_K, 512], f16, name="qT_0"),
                proj_pool.tile([D_K, 512], f16, name="qT_1"),
                proj_pool.tile([D_K, 1984], f16, name="qT_rest"),
            ]

            def kt_slice(j):
                """rhs AP for the 512-wide kT chunk j."""
                if j < 2:
                    return kTg[j][:]
                if j < 4:
                    return kTg[2][:, (j - 2) * 512 : (j - 1) * 512]
                return kTg[3][:, (j - 4) * 512 : (j - 3) * 512]

            def q_lhs(i, mt):
                """lhsT AP for m-tile i."""
                if i < 4:
                    g, base = 0, 0
                elif i < 8:
                    g, base = 1, 512
                else:
                    g, base = 2, 1024
                off = i * P - base
                return qTg[g][:, off : off + mt]

            def load_pair(dst, dram, c0, width):
                src_ap = dram.ap()[:, c0 : c0 + width].rearrange("(a p) c -> p a c", p=P)
                nc.sync.dma_start(dst[:], src_ap)

            def project(dst, w2, srcT0, srcT1, width, nm):
                """dst[:, :width] = (w^T x srcT)[32, width] via psum, 512-wide mms.
                Copy must run on DVE: GPSIMD cannot read PSUM."""
                ps = psum_tile(nm)
                for u in range(0, width, 512):
                    uw = min(512, width - u)
                    nc.tensor.matmul(
                        ps[:D_K, u : u + uw], w2[0], srcT0[:, u : u + uw],
                        start=True, stop=False,
                    )
                    nc.tensor.matmul(
                        ps[:D_K, u : u + uw], w2[1], srcT1[:, u : u + uw],
                        start=False, stop=True,
                    )
                nc.vector.tensor_copy(dst[:], ps[:D_K, :width])

            def score_mm(ps, mt, i, half):
                """4 matmuls filling ps[:mt, :2048] for m-tile i, kT chunks half*4..+3."""
                lhs = q_lhs(i, mt)
                for u in range(4):
                    nc.tensor.matmul(
                        ps[:mt, u * 512 : (u + 1) * 512], lhs, kt_slice(half * 4 + u),
                        start=True, stop=True,
                    )

            def exp_chunk(ps, exp_t, mt, dcol, scol, width, accum=None):
                nc.scalar.activation(
                    exp_t[:mt, dcol : dcol + width],
                    ps[:mt, scol : scol + width],
                    EXP, scale=SCALE,
                    accum_out=None if accum is None else accum[:mt],
                )

            def store_half(i, exp_t, mt, half):
                nc.sync.dma_start(
                    attn.ap()[i * P : i * P + mt, half * 2048 : (half + 1) * 2048],
                    exp_t[:mt, half * 2048 : (half + 1) * 2048],
                )

            def row_sums(i, exp_t, mt, pool_first):
                """Binary-tree row sums of exp_t into sums_sb[:, i]."""
                sc = scrp.tile([P, 2048], f16, tag="scr", name=f"sc{i}")
                eng = nc.gpsimd if pool_first else nc.vector
                eng.tensor_tensor(sc[:mt, :2048], exp_t[:mt, :2048], exp_t[:mt, 2048:], op=ADD)
                w = 1024
                while w >= 64:
                    nc.vector.tensor_tensor(sc[:mt, :w], sc[:mt, :w], sc[:mt, w : 2 * w], op=ADD)
                    w //= 2
                nc.vector.tensor_reduce(
                    sums_sb[:mt, i : i + 1], sc[:mt, :64], axis=mybir.AxisListType.X, op=ADD
                )

            # ================= emission =================
            # boot carries everything the first 512-wide exp chunk needs; all
            # remaining loads are queued immediately after it (stores only
            # show up ~8us in, so the load stream owns the DMA device early).
            load_pair(keyP[0], keyT_d, *KP[0])
            load_pair(keyP[1], keyT_d, *KP[1])
            load_pair(keyP[2], keyT_d, *KP[2])
            load_pair(keyP[3], keyT_d, *KP[3])
            load_pair(queryP[0], queryT_d, *QP[0])
            load_pair(queryP[1], queryT_d, *QP[1])
            load_pair(queryP[2], queryT_d, *QP[2])

            # projection mm-pairs go out before the score mms that consume
            # them so they never sit behind a blocked score mm in the queues
            project(qTg[0], wq, bq[0], bq[1], 512, "pq0")
            project(kTg[0], wk, keyP[0][:, 0, :], keyP[0][:, 1, :], 512, "pk0")
            project(kTg[1], wk, keyP[1][:, 0, :], keyP[1][:, 1, :], 512, "pk1")

            # tile 0 A-half in fine exp chunks that chase the arriving kT
            exp_tiles = {}
            exp_tiles[0] = expp.tile([P, CONCEPT_NUM], f16, tag="exp", name="exp0")
            ps_a0 = psum_tile("psA0a")
            lhs0 = q_lhs(0, P)
            nc.tensor.matmul(ps_a0[:P, 0:512], lhs0, kt_slice(0), start=True, stop=True)
            exp_chunk(ps_a0, exp_tiles[0], P, 0, 0, 512)
            nc.tensor.matmul(ps_a0[:P, 512:1024], lhs0, kt_slice(1), start=True, stop=True)
            exp_chunk(ps_a0, exp_tiles[0], P, 512, 512, 512)

            project(kTg[2], wk, keyP[2][:, 0, :], keyP[2][:, 1, :], 1024, "pk23")
            ps_a0b = psum_tile("psA0b")
            nc.tensor.matmul(ps_a0b[:P, 0:512], lhs0, kt_slice(2), start=True, stop=True)
            nc.tensor.matmul(ps_a0b[:P, 512:1024], lhs0, kt_slice(3), start=True, stop=True)
            exp_chunk(ps_a0b, exp_tiles[0], P, 1024, 0, 1024)
            store_half(0, exp_tiles[0], P, 0)

            # phase 1: A-halves of tiles 1..N1-1 (need only kT 0..3); the
            # kT 4..7 projection slots between them so its psum alloc does
            # not gate a phase-1 tile
            for i in range(1, N1):
                exp_tiles[i] = expp.tile([P, CONCEPT_NUM], f16, tag="exp", name=f"exp{i}")
                ps = psum_tile(f"psA{i}")
                score_mm(ps, m_tiles[i], i, 0)
                exp_chunk(ps, exp_tiles[i], m_tiles[i], 0, 0, 2048)
                store_half(i, exp_tiles[i], m_tiles[i], 0)
                if i == 1:
                    project(qTg[1], wq, queryP[0][:, 0, :], queryP[0][:, 1, :], 512, "pq1")
                elif i == 2:
                    project(kTg[3], wk, keyP[3][:, 0, :], keyP[3][:, 1, :], 2048, "pk47")

            # phase 2: B-halves + row sums of tiles 0..N1-1
            for i in range(N1):
                ps = psum_tile(f"psB{i}")
                score_mm(ps, m_tiles[i], i, 1)
                exp_chunk(ps, exp_tiles[i], m_tiles[i], 2048, 0, 2048)
                store_half(i, exp_tiles[i], m_tiles[i], 1)
                row_sums(i, exp_tiles[i], m_tiles[i], pool_first=(i % 2 == 0))
                if i == 0:
                    ps_qr = psum_tile("pqrest")
                    for u in range(0, 1984, 512):
                        uw = min(512, 1984 - u)
                        sP, sc0 = (queryP[1], u) if u < 1024 else (queryP[2], u - 1024)
                        nc.tensor.matmul(ps_qr[:D_K, u : u + uw], wq[0], sP[:, 0, sc0 : sc0 + uw], start=True, stop=False)
                        nc.tensor.matmul(ps_qr[:D_K, u : u + uw], wq[1], sP[:, 1, sc0 : sc0 + uw], start=False, stop=True)
                    nc.vector.tensor_copy(qTg[2][:], ps_qr[:D_K, :1984])

            # phase 3: steady-state pipeline, tiles N1..22
            for i in range(N1, n_mt - 1):
                mt = m_tiles[i]
                exp_t = expp.tile([P, CONCEPT_NUM], f16, tag="exp", name=f"exp{i}")
                ps_a = psum_tile(f"psA{i}")
                score_mm(ps_a, mt, i, 0)
                exp_chunk(ps_a, exp_t, mt, 0, 0, 2048)
                store_half(i, exp_t, mt, 0)
                ps_b = psum_tile(f"psB{i}")
                score_mm(ps_b, mt, i, 1)
                exp_chunk(ps_b, exp_t, mt, 2048, 0, 2048)
                store_half(i, exp_t, mt, 1)
                row_sums(i, exp_t, mt, pool_first=(i % 2 == 0 and i < POOL_LAST))

            # tail tile (56 rows): ACT accumulates the row sums itself and the
            # B-half goes out as two 1024-wide chunks so the last store is
            # small; the end-of-stream tail is just that store + sums.
            i = n_mt - 1
            mt = m_tiles[i]
            exp_t = expp.tile([P, CONCEPT_NUM], f16, tag="exp", name=f"exp{i}")
            s_a = stats.tile([P, 1], f32, tag="acca", name="s_a")
            s_b = stats.tile([P, 1], f32, tag="accb", name="s_b")
            s_c = stats.tile([P, 1], f32, tag="accc", name="s_c")
            ps_a = psum_tile(f"psA{i}")
            score_mm(ps_a, mt, i, 0)
            exp_chunk(ps_a, exp_t, mt, 0, 0, 2048, accum=s_a)
            store_half(i, exp_t, mt, 0)
            ps_b = psum_tile(f"psB{i}")
            score_mm(ps_b, mt, i, 1)
            exp_chunk(ps_b, exp_t, mt, 2048, 0, 2048, accum=s_b)
            store_half(i, exp_t, mt, 1)
            nc.vector.tensor_tensor(sums_sb[:mt, i : i + 1], s_a[:mt], s_b[:mt], op=ADD)
            nc.gpsimd.dma_start(sums_d.ap(), sums_sb[:])

    nc.compile()
    return nc


def _get_module():
    if "nc" not in _BUILD_CACHE:
        _BUILD_CACHE["nc"] = _build_module()
    return _BUILD_CACHE["nc"]


def kernel(qt, query, key_emb, w_q, w_k):
    from concourse.bass_utils import run_bass_kernel_spmd

    qt = np.asarray(qt)
    queryT16 = np.zeros((INPUT_DIM, MPAD), dtype=np.float16)
    queryT16[:, :MASK_NUM] = np.asarray(query, dtype=np.float16).T
    keyT16 = np.ascontiguousarray(np.asarray(key_emb, dtype=np.float16).T)
    w_q = np.asarray(w_q, dtype=np.float16)
    w_k = np.asarray(w_k, dtype=np.float16)

    nc = _get_module()
    in_maps = []
    for h in range(N_HEAD):
        boot = np.empty((P, BOOTW), dtype=np.float16)
        wq_h = w_q[:, h * D_K : (h + 1) * D_K]
        wk_h = w_k[:, h * D_K : (h + 1) * D_K]
        boot[:, 0:D_K] = wq_h[:P]
        boot[:, D_K : 2 * D_K] = wq_h[P:]
        boot[:, 2 * D_K : 3 * D_K] = wk_h[:P]
        boot[:, 3 * D_K : 4 * D_K] = wk_h[P:]
        O = 4 * D_K
        boot[:, O : O + 512] = queryT16[:P, :512]
        boot[:, O + 512 : O + 1024] = queryT16[P:, :512]
        in_maps.append({"boot": boot, "queryT": queryT16, "keyT": keyT16})
    res = run_bass_kernel_spmd(nc, in_maps, core_ids=list(range(N_HEAD)))

    rows = qt.astype(np.int64)
    full = np.zeros((N_HEAD, CONCEPT_NUM, CONCEPT_NUM), dtype=np.float32)
    for h in range(N_HEAD):
        r = res.results[h]
        # sums[p, t] is the row-sum of mask row t*128 + p
        inv = 1.0 / r["sums"].T.reshape(-1)[:MASK_NUM].astype(np.float32)
        full[h, rows, :] = r["attn"].astype(np.float32) * inv[:, None]
    return full
